# revision 44
# baseline (speedup 1.0000x reference)
"""CKConv (SIREN continuous-kernel conv) Trainium2 Bass kernel.

Math: the reference evaluates a SIREN net at rel[e,s] = t[s] - t_eval[e],
masks causally (rel <= 0), and contracts with x:
    out[e,g] = sum_{s<=e, c} K(rel[e,s])[g,c] * x[s,c]
Both t and t_eval are arange(512)/512, so rel[e,s] = (s-e)/512 exactly in
fp32 -- it depends only on the lag j = e - s in [0, 511].  The net therefore
only needs evaluation at 512 distinct inputs rel_j = -j/512, and the output
is a causal Toeplitz conv:
    out[e] = sum_{j=0}^{e} K'[j] @ x[e-j],   K'[j] in R^{16x16}.

Sharding: 8 cores split the contraction by input channel: core m owns
channels {2m, 2m+1} x all 4 lag blocks of 128.  Host builds Hankel tiles
H[(jb,ci)][p, e] = xpad[e - 128*jb - p, c] (pure data movement of x, bf16),
sums the per-core partial (16, 512) outputs and transposes -> (512, 16).

Per-core device program (final):
  * "v-layout": partition p = 32*jg + i packs 4 lag-groups x 32 hidden units.
  * layer 1's argument depends only on the known time grid -> the host
    ships v1 = u1 - round(u1) directly; device layer 1 is a single ACT.
  * layer 2 range reduction in 3 DVE ops: t1 = (mm2+c2)+M rounds to M+k
    (M = 1.5*2^23), k = t1-M (exact), d2 = mm2-k; the ACT then computes
    sin(2pi*d2 + omega*b2) == sin(2pi*(u-k)) -- the +c2 is recombined via
    the per-partition ACT bias so the final arg stays in [-pi, pi].
  * layers 2/3: 4 concurrent 32x32 tile_position matmuls in fp16 (1 PE
    cycle/row vs fp32's 4; fp16's 10-bit mantissa keeps the omega-amplified
    phase error ~3e-3 -- bf16 would not).  Layer 3 writes one 4-bank PSUM
    tile; a single strided-AP tensor_tensor adds b3 (stride-0 broadcast)
    and converts to bf16.
  * conv in bf16 (1 PE cycle/row vs 4 for fp32), split into two
    accumulation groups by e-range ([0,384) and [384,512)) so the first
    group's PSUM->SBUF copy and DMA overlap the second group's matmuls.
  * DMA: descriptor generation paces at ~9ns/row globally, so transfers
    are few and wide; params (gates the SIREN) is issued first and a tiny
    spacer DMA delays the hank flood so params rows never straggle.  The
    hank moves as two halves (round A first) so the conv's round-A matmuls
    unblock ~1us earlier; the first out-DMA is issued from the scalar
    engine so the second's trigger is not serialized behind it on sync.
  * PE DVFS: the PE streams at 1.2GHz until it has been busy ~3-4us, then
    ramps toward 2.4GHz.  Dummy bf16 matmuls into a spare PSUM bank fill
    the params-DMA wait, and small fillers anchored by data dependencies
    on h1/kr/d2/h2 (so the scheduler cannot float them forward) bridge
    the DVE/ACT-chain bubbles -- the conv then runs at full clock.
"""

import numpy as np

import concourse.mybir as mybir
import concourse.tile as tile
from concourse import bacc
from concourse.bass_utils import run_bass_kernel_spmd

F32 = mybir.dt.float32
F16 = mybir.dt.float16
BF16 = mybir.dt.bfloat16
L = 512          # sequence length == L_eval
CIN = 16
COUT = 16
H = 32           # SIREN hidden
OMEGA = 32.5
NCORES = 8
NJB = 4          # lag blocks of 128
PAD = 512        # zero padding rows in front of x for the Hankel build
TWO_PI = 2.0 * np.pi
MAGIC = float(1.5 * 2.0**23)  # fp32 add/sub rounds to nearest integer

# packed param layout (128, PCOLS), partition p = 32*jg + i
P_V1 = 0       # [:, 0:128]    v1[p, jj] = u1 - round(u1), u1 = a1*rel + c1
P_W2 = 128     # [:, 128:160]  w2v[32jg+i, o] = (omega/2pi) * W2[o, i] (x4)
P_C2 = 160     # [:, 160]      c2[i] = (omega/2pi)*b2[i] (x4)
P_C2B = 161    # [:, 161]      c2b[i] = omega*b2[i] (ACT bias) (x4)
P_W3 = 162     # [:, 162:194]  w3v[32b+o, m] = W3[colsel[m], o] (x4)
P_B3 = 194     # [:, 194:226]  b3v[p, m] = b3[colsel[m]] (bcast x4 via AP)
PCOLS = 226

# Hankel chunks, causally trimmed: chunk (b, ci) covers e in [128b, 512)
CH_N = [L - 128 * b for b in range(NJB)]          # 512, 384, 256, 128
CH_OFF_A = [sum(CH_N[:b]) for b in range(NJB)]    # round A (ci=0) offsets
HCOLS_HALF = sum(CH_N)                            # 1280
HCOLS = 2 * HCOLS_HALF

_CACHE = {}


def _build_module():
    # Bacc (not raw Bass): its compile() splits multi-sem sync waits into
    # event-semaphore instructions -- walrus allows only 1 wait per inst.
    nc = bacc.Bacc("TRN2", target_bir_lowering=False, debug=False)

    params_d = nc.dram_tensor("params", [128, PCOLS], F32, kind="ExternalInput")
    # Hankel tiles packed along free dim, causally trimmed, bf16; round A
    # (ci=0) chunks first.  chunk (b, ci): cols [ci*HCOLS_HALF + CH_OFF_A[b],
    # +CH_N[b]); H[p, e'] = xpad[e' - 128b - p, c] for e' in [128b, 512)
    hank_d = nc.dram_tensor("hank", [128, HCOLS], BF16, kind="ExternalInput")
    out_d = nc.dram_tensor("out", [COUT, L], F32, kind="ExternalOutput")

    with tile.TileContext(nc) as tc:
        with (
            tc.tile_pool(name="sb", bufs=1) as sb,
            tc.tile_pool(name="ps", bufs=1, space="PSUM") as ps,
        ):
            # DMA order: the SIREN-gating params first (small, finishes while
            # the hank rings spin up), then the hank halves, then b3 (only
            # needed ~3us later at the bias add).  Separate tiles so the
            # dependency tracking doesn't serialize on the whole params DMA.
            # DMA descriptor generation paces at ~9ns/row globally, so fewer
            # and larger transfers win: one params DMA (gates the SIREN,
            # issued first) and one hank DMA.
            # params in two transfers: v1 (gates layer 1) first, the rest
            # second -- the second transfer doubles as the spacer that keeps
            # the hank descriptor flood off the v1 rows' tail
            pt1 = sb.tile([128, P_W2], F32)
            nc.sync.dma_start(pt1[:], params_d[:, 0:P_W2])
            pt2 = sb.tile([128, PCOLS - P_W2], F32)
            nc.sync.dma_start(pt2[:], params_d[:, P_W2:PCOLS])
            # hank in two halves (round A then round B): the conv consumes
            # round-A chunks first, so it can start ~1us earlier than a
            # single hank transfer would allow
            ht = sb.tile([128, HCOLS], BF16)
            nc.sync.dma_start(ht[:, 0:HCOLS_HALF], hank_d[:, 0:HCOLS_HALF])
            nc.sync.dma_start(
                ht[:, HCOLS_HALF:HCOLS], hank_d[:, HCOLS_HALF:HCOLS]
            )

            v1 = pt1[:, P_V1 : P_V1 + 128]
            w2v = pt2[:, 0 : H]
            c2 = pt2[:, P_C2 - P_W2 : P_C2 - P_W2 + 1]
            c2b = pt2[:, P_C2B - P_W2 : P_C2B - P_W2 + 1]
            w3v = pt2[:, P_W3 - P_W2 : P_W3 - P_W2 + 2 * COUT]

            BANK = 512  # fp32 elements per PSUM bank

            # ---- PE p-state warmup: the PE needs ~3us of continuous work
            # to reach its 2.4GHz p-state (it streams at 1.2GHz before
            # that).  Fill the params-DMA wait and the SIREN DVE/ACT gaps
            # with dummy bf16 matmuls into the spare PSUM bank so the conv
            # (the big stream) runs at full clock.
            wsrc = sb.tile([128, 256], BF16)
            nc.gpsimd.memset(wsrc[:], 0.0)
            wps = ps.tile([COUT, BANK], F32, name="wps", tag="wps")

            def warm(src_ap, n_rows):
                nc.tensor.matmul(
                    wps[0:COUT, 0:n_rows], src_ap[:, 0:COUT],
                    src_ap[:, 0:n_rows], start=True, stop=True,
                )

            for _ in range(11):
                warm(wsrc, 256)

            # fp16 copies of the tiny layer-2/3 weights: fp16 matmuls
            # stream at 1 PE cycle/row vs fp32's 4 (fp16's 10-bit mantissa
            # keeps the omega-amplified phase error ~2.5e-3 rad; bf16 would
            # not).  Pool engine does the converts off the critical path.
            w2h = sb.tile([128, H], F16)
            nc.gpsimd.tensor_copy(w2h[:], w2v)
            w3h = sb.tile([128, 2 * COUT], F16)
            nc.gpsimd.tensor_copy(w3h[:], w3v)

            # ---- SIREN layer 1: h1 = sin(2pi*v1 - pi) on all 128 partitions
            h1 = sb.tile([128, 128], F16)
            nc.scalar.activation(
                h1[:], v1, mybir.ActivationFunctionType.Sin, scale=TWO_PI
            )

            # ---- SIREN layer 2: 4 concurrent 32x32 tile_position matmuls
            # (W2 pre-scaled by omega/2pi on host), then one fused
            # (+c2' mod 1) on DVE and the Sin ACT.
            mm2 = ps.tile([128, 128], F32, tag="mm2")
            for jg in range(NJB):
                s = slice(32 * jg, 32 * jg + 32)
                nc.tensor.matmul(
                    mm2[s, :], w2h[s, :], h1[s, :],
                    start=True, stop=True, tile_position=(32 * jg, 32 * jg),
                )
            # gap fillers: anchored by a data dependency on h1/kr so the
            # scheduler cannot float them ahead of mm2; under-sized so they
            # end before layer 3 becomes ready.  They keep the PE's DVFS
            # ramp alive through the DVE/ACT chain.
            warm(h1, 64)
            warm(h1, 64)
            warm(h1, 64)
            warm(h1, 64)
            t1 = sb.tile([128, 128], F32)
            nc.vector.tensor_scalar(
                t1[:], mm2[:], c2, MAGIC,
                mybir.AluOpType.add, mybir.AluOpType.add,
            )
            kr = sb.tile([128, 128], F32)
            nc.vector.tensor_scalar(
                kr[:], t1[:], MAGIC, None, mybir.AluOpType.subtract
            )
            warm(kr, 64)
            d2 = sb.tile([128, 128], F32)
            nc.vector.tensor_sub(d2[:], mm2[:], kr[:])
            warm(d2, 48)
            h2 = sb.tile([128, 128], F16)
            nc.scalar.activation(
                h2[:], d2[:], mybir.ActivationFunctionType.Sin,
                bias=c2b, scale=TWO_PI,
            )

            # ---- layer 3: K[jj, m] per lag block b -- 4 concurrent matmuls
            # into one 4-bank PSUM tile (block b at bank b, first 32 cols),
            # then a single strided-AP add of b3 converting to bf16.
            kps = ps.tile([128, NJB * BANK], F32, tag="kps")
            for b in range(NJB):
                s = slice(32 * b, 32 * b + 32)
                nc.tensor.matmul(
                    kps[:, b * BANK : b * BANK + 2 * COUT],
                    h2[s, :], w3h[s, :],
                    start=True, stop=True, tile_position=(32 * b, 0),
                )
            warm(h2, 64)
            warm(h2, 32)
            warm(h2, 128)
            warm(h2, 128)
            # bias add in two halves with SEPARATE destination tiles (tile-
            # granular deps -- slicing one tile races the conv's LDWEIGHTS
            # prefetch on hardware): the conv's first chunks need only
            # blocks 0-1, so they start after the first half.
            b3v2 = (
                pt2[:, P_B3 - P_W2 : P_B3 - P_W2 + 2 * COUT]
                .unsqueeze(1)
                .broadcast_to([128, 2, 2 * COUT])
            )
            kview = kps[:].rearrange("p (b n) -> p b n", b=NJB)
            ksbA = sb.tile([128, 2 * 2 * COUT], BF16)
            nc.vector.tensor_add(
                ksbA[:].rearrange("p (b n) -> p b n", b=2),
                kview[:, 0:2, 0 : 2 * COUT], b3v2,
            )
            ksbB = sb.tile([128, 2 * 2 * COUT], BF16)
            nc.vector.tensor_add(
                ksbB[:].rearrange("p (b n) -> p b n", b=2),
                kview[:, 2:NJB, 0 : 2 * COUT], b3v2,
            )

            # ---- causal conv, split into two accumulation groups by e-half
            # so the first half's PSUM->SBUF copy + DMA overlap the second
            # half's matmuls.  Each group's first chunk covers the whole
            # group range (opens it) and its last chunk does too (closes it).
            # chunk (b, ci) covers e in [128b, 512) (causal trimming); the
            # e-half [e0, e1) slice of it is chunk-cols [e0-128b, e1-128b).
            th = sb.tile([COUT, L], F32)
            halves = [(0, 384, [(0, 0), (0, 1), (0, 2), (1, 2), (1, 1), (1, 0)]),
                      (384, 512, [(0, 0), (0, 1), (0, 2), (0, 3),
                                  (1, 3), (1, 2), (1, 1), (1, 0)])]
            for e0, e1, grp in halves:
                # full-bank tile so the two groups never share a PSUM bank
                # (sharing would serialize group B behind group A's copy)
                vp = ps.tile([COUT, BANK], F32, name=f"vp{e0}", tag=f"vp{e0}")
                for idx, (ci, b) in enumerate(grp):
                    kt = ksbA if b < 2 else ksbB
                    lhs = kt[:, (b % 2) * 2 * COUT + ci * COUT
                             : (b % 2) * 2 * COUT + (ci + 1) * COUT]
                    off = ci * HCOLS_HALF + CH_OFF_A[b] + max(e0 - 128 * b, 0)
                    lo = max(e0, 128 * b)
                    nc.tensor.matmul(
                        vp[0:COUT, lo - e0 : e1 - e0],
                        lhs, ht[:, off : off + (e1 - lo)],
                        start=(idx == 0), stop=(idx == len(grp) - 1),
                    )
                nc.vector.tensor_copy(th[:, e0:e1], vp[0:COUT, 0 : e1 - e0])
                eng = nc.scalar if e0 == 0 else nc.sync
                eng.dma_start(out_d[:, e0:e1], th[:, e0:e1])

    nc.compile()
    return nc


def _host_prep(inputs):
    """Fold params and build per-core in_maps (numpy)."""
    import ml_dtypes

    x = np.asarray(inputs["x"], np.float32)
    t = np.asarray(inputs["t"], np.float32)
    t_eval = np.asarray(inputs["t_eval"], np.float32)
    v1 = np.asarray(inputs["v1"], np.float32)
    g1 = np.asarray(inputs["g1"], np.float32)
    b1 = np.asarray(inputs["b1"], np.float32)
    v2 = np.asarray(inputs["v2"], np.float32)
    g2 = np.asarray(inputs["g2"], np.float32)
    b2 = np.asarray(inputs["b2"], np.float32)
    W3 = np.asarray(inputs["W3"], np.float32)
    b3 = np.asarray(inputs["b3"], np.float32)

    # weight norm (fp32, matching reference)
    W1 = (g1[:, None] * v1 / np.linalg.norm(v1, axis=1, keepdims=True))[:, 0]
    W2 = g2[:, None] * v2 / np.linalg.norm(v2, axis=1, keepdims=True)

    # rel_j = t[0] - t_eval[j]  (== -j/512 exactly on the arange grid)
    rel = (np.float32(t[0]) - t_eval).astype(np.float64)

    s = np.float64(OMEGA) / TWO_PI
    a1 = s * W1.astype(np.float64)
    c1 = s * b1.astype(np.float64)
    # layer-1 argument in cycles, range-reduced on host (pure function of
    # the known time grid + params): sin(2pi*v1) == sin(2pi*u1)
    u1 = a1[:, None] * rel[None, :] + c1[:, None]             # (H, 512)
    v1c = (u1 - np.round(u1)).astype(np.float32)              # (H, 512)

    c2 = (s * b2.astype(np.float64)).astype(np.float32)
    c2b = (np.float64(OMEGA) * b2.astype(np.float64)).astype(np.float32)
    w2s = (s * W2.astype(np.float64)).astype(np.float32)      # (H, H)

    xpad = np.zeros((PAD + L, CIN), np.float32)
    xpad[PAD:] = x

    # shared parts of the packed params (128, PCOLS)
    base = np.zeros((128, PCOLS), np.float32)
    # v-layout: partition p = 32*jg + i covers lags 128jg..128jg+127
    base[:, P_V1 : P_V1 + 128] = (
        v1c.reshape(H, NJB, 128).transpose(1, 0, 2).reshape(128, 128)
    )
    base[:, P_C2] = np.tile(c2, NJB)
    base[:, P_C2B] = np.tile(c2b, NJB)
    base[:, P_W2 : P_W2 + H] = np.tile(w2s.T, (NJB, 1))

    in_maps = []
    for m in range(NCORES):
        cols = []
        for ci in range(2):
            c = 2 * m + ci
            cols.extend(g * CIN + c for g in range(COUT))
        params = base.copy()
        params[:, P_W3 : P_W3 + 2 * COUT] = np.tile(W3[cols, :].T, (NJB, 1))
        params[:, P_B3 : P_B3 + 2 * COUT] = np.broadcast_to(b3[cols], (128, 2 * COUT))

        hank = np.zeros((128, HCOLS), ml_dtypes.bfloat16)
        for ci in range(2):
            c = 2 * m + ci
            # H[p, e] = x[e - 128*b - p, c] (0 when index < 0)
            w = np.lib.stride_tricks.sliding_window_view(xpad[:, c], L)
            for b in range(NJB):
                rows = PAD - 128 * b - np.arange(128)
                off = ci * HCOLS_HALF + CH_OFF_A[b]
                hank[:, off : off + CH_N[b]] = w[rows][:, 128 * b : L].astype(
                    ml_dtypes.bfloat16
                )
        in_maps.append({"params": params, "hank": hank})
    return in_maps


def kernel(**inputs) -> np.ndarray:
    if "nc" not in _CACHE:
        _CACHE["nc"] = _build_module()
    nc = _CACHE["nc"]
    in_maps = _host_prep(inputs)
    res = run_bass_kernel_spmd(nc, in_maps, list(range(NCORES)))
    partial = np.zeros((COUT, L), np.float64)
    for r in res.results:
        partial += r["out"].astype(np.float64)
    return partial.T.astype(np.float32)


# revision 45
# speedup vs baseline: 1.1984x; 1.1984x over previous
"""CKConv (SIREN continuous-kernel conv) Trainium2 Bass kernel.

Math: the reference evaluates a SIREN net at rel[e,s] = t[s] - t_eval[e],
masks causally (rel <= 0), and contracts with x:
    out[e,g] = sum_{s<=e, c} K(rel[e,s])[g,c] * x[s,c]
Both t and t_eval are arange(512)/512, so rel[e,s] = (s-e)/512 exactly in
fp32 -- it depends only on the lag j = e - s in [0, 511].  The net therefore
only needs evaluation at 512 distinct inputs rel_j = -j/512, and the output
is a causal Toeplitz conv:
    out[e] = sum_{j=0}^{e} K'[j] @ x[e-j],   K'[j] in R^{16x16}.

Sharding: 8 cores split the contraction by input channel: core m owns
channels {2m, 2m+1} x all 4 lag blocks of 128.  Host builds Hankel tiles
H[(jb,ci)][p, e] = xpad[e - 128*jb - p, c] (pure data movement of x, bf16),
sums the per-core partial (16, 512) outputs and transposes -> (512, 16).

Per-core device program (final):
  * "v-layout": partition p = 32*jg + i packs 4 lag-groups x 32 hidden units.
  * layer 1's argument depends only on the known time grid -> the host
    ships v1 = u1 - round(u1) directly; device layer 1 is a single ACT.
  * layer 2 range reduction in 3 DVE ops: t1 = (mm2+c2)+M rounds to M+k
    (M = 1.5*2^23), k = t1-M (exact), d2 = mm2-k; the ACT then computes
    sin(2pi*d2 + omega*b2) == sin(2pi*(u-k)) -- the +c2 is recombined via
    the per-partition ACT bias so the final arg stays in [-pi, pi].
  * layers 2/3: 4 concurrent 32x32 tile_position matmuls in fp16 (1 PE
    cycle/row vs fp32's 4; fp16's 10-bit mantissa keeps the omega-amplified
    phase error ~3e-3 -- bf16 would not).  Layer 3 writes one 4-bank PSUM
    tile; a single strided-AP tensor_tensor adds b3 (stride-0 broadcast)
    and converts to bf16.
  * conv in bf16 (1 PE cycle/row vs 4 for fp32), split into two
    accumulation groups by e-range ([0,384) and [384,512)) so the first
    group's PSUM->SBUF copy and DMA overlap the second group's matmuls.
  * DMA: descriptor generation paces at ~9ns/row globally, so transfers
    are few and wide; params (gates the SIREN) is issued first and a tiny
    spacer DMA delays the hank flood so params rows never straggle.  The
    hank moves as two halves (round A first) so the conv's round-A matmuls
    unblock ~1us earlier; the first out-DMA is issued from the scalar
    engine so the second's trigger is not serialized behind it on sync.
  * PE DVFS: the PE streams at 1.2GHz until it has been busy ~3-4us, then
    ramps toward 2.4GHz.  Dummy bf16 matmuls into a spare PSUM bank fill
    the params-DMA wait, and small fillers anchored by data dependencies
    on h1/kr/d2/h2 (so the scheduler cannot float them forward) bridge
    the DVE/ACT-chain bubbles -- the conv then runs at full clock.
"""

import numpy as np

import concourse.mybir as mybir
import concourse.tile as tile
from concourse import bacc
from concourse.bass_utils import run_bass_kernel_spmd

F32 = mybir.dt.float32
F16 = mybir.dt.float16
BF16 = mybir.dt.bfloat16
L = 512          # sequence length == L_eval
CIN = 16
COUT = 16
H = 32           # SIREN hidden
OMEGA = 32.5
NCORES = 8
NJB = 4          # lag blocks of 128
PAD = 512        # zero padding rows in front of x for the Hankel build
TWO_PI = 2.0 * np.pi
MAGIC = float(1.5 * 2.0**23)  # fp32 add/sub rounds to nearest integer

# packed param layout (128, PCOLS), partition p = 32*jg + i.  The layer-2/3
# weights are fp16 BITS packed into f32 columns (the device bitcasts the
# SBUF view) so no on-device convert sits between the params DMA and mm2.
P_V1 = 0       # [:, 0:128]    v1[p, jj] = u1 - round(u1), u1 = a1*rel + c1
P_W2 = 128     # [:, 128:144]  fp16 bits: w2h[32jg+i, o] = (om/2pi)W2[o, i]
P_W3 = 144     # [:, 144:160]  fp16 bits: w3h[32b+o, m] = W3[colsel[m], o]
P_C2 = 160     # [:, 160]      c2[i] = (omega/2pi)*b2[i] (x4)
P_C2B = 161    # [:, 161]      c2b[i] = omega*b2[i] (ACT bias) (x4)
P_B3 = 162     # [:, 162:194]  b3v[p, m] = b3[colsel[m]] (bcast x4 via AP)
PCOLS = 194

# Hankel chunks, causally trimmed: chunk (b, ci) covers e in [128b, 512)
CH_N = [L - 128 * b for b in range(NJB)]          # 512, 384, 256, 128
CH_OFF_A = [sum(CH_N[:b]) for b in range(NJB)]    # round A (ci=0) offsets
HCOLS_HALF = sum(CH_N)                            # 1280
HCOLS = 2 * HCOLS_HALF

_CACHE = {}


def _build_module():
    # Bacc (not raw Bass): its compile() splits multi-sem sync waits into
    # event-semaphore instructions -- walrus allows only 1 wait per inst.
    nc = bacc.Bacc("TRN2", target_bir_lowering=False, debug=False)

    params_d = nc.dram_tensor("params", [128, PCOLS], F32, kind="ExternalInput")
    # Hankel tiles packed along free dim, causally trimmed, bf16; round A
    # (ci=0) chunks first.  chunk (b, ci): cols [ci*HCOLS_HALF + CH_OFF_A[b],
    # +CH_N[b]); H[p, e'] = xpad[e' - 128b - p, c] for e' in [128b, 512)
    hank_d = nc.dram_tensor("hank", [128, HCOLS], BF16, kind="ExternalInput")
    out_d = nc.dram_tensor("out", [COUT, L], F32, kind="ExternalOutput")

    with tile.TileContext(nc) as tc:
        with (
            tc.tile_pool(name="sb", bufs=1) as sb,
            tc.tile_pool(name="ps", bufs=1, space="PSUM") as ps,
        ):
            # DMA order: the SIREN-gating params first (small, finishes while
            # the hank rings spin up), then the hank halves, then b3 (only
            # needed ~3us later at the bias add).  Separate tiles so the
            # dependency tracking doesn't serialize on the whole params DMA.
            # DMA descriptor generation paces at ~9ns/row globally, so fewer
            # and larger transfers win: one params DMA (gates the SIREN,
            # issued first) and one hank DMA.
            # params in two transfers: v1 (gates layer 1) first, the rest
            # second -- the second transfer doubles as the spacer that keeps
            # the hank descriptor flood off the v1 rows' tail
            pt1 = sb.tile([128, P_W2], F32)
            nc.sync.dma_start(pt1[:], params_d[:, 0:P_W2])
            pt2 = sb.tile([128, PCOLS - P_W2], F32)
            nc.sync.dma_start(pt2[:], params_d[:, P_W2:PCOLS])
            # hank in two halves (round A then round B): the conv consumes
            # round-A chunks first, so it can start ~1us earlier than a
            # single hank transfer would allow
            ht = sb.tile([128, HCOLS], BF16)
            nc.sync.dma_start(ht[:, 0:HCOLS_HALF], hank_d[:, 0:HCOLS_HALF])
            nc.sync.dma_start(
                ht[:, HCOLS_HALF:HCOLS], hank_d[:, HCOLS_HALF:HCOLS]
            )

            v1 = pt1[:, P_V1 : P_V1 + 128]
            w2h = pt2[:, 0:16].bitcast(F16)
            w3h = pt2[:, P_W3 - P_W2 : P_W3 - P_W2 + 16].bitcast(F16)
            c2 = pt2[:, P_C2 - P_W2 : P_C2 - P_W2 + 1]
            c2b = pt2[:, P_C2B - P_W2 : P_C2B - P_W2 + 1]

            BANK = 512  # fp32 elements per PSUM bank

            # ---- PE p-state warmup: the PE needs ~3us of continuous work
            # to reach its 2.4GHz p-state (it streams at 1.2GHz before
            # that).  Fill the params-DMA wait and the SIREN DVE/ACT gaps
            # with dummy bf16 matmuls into the spare PSUM bank so the conv
            # (the big stream) runs at full clock.
            wsrc = sb.tile([128, 256], BF16)
            nc.gpsimd.memset(wsrc[:], 0.0)
            wps = ps.tile([COUT, BANK], F32, name="wps", tag="wps")

            def warm(src_ap, n_rows):
                nc.tensor.matmul(
                    wps[0:COUT, 0:n_rows], src_ap[:, 0:COUT],
                    src_ap[:, 0:n_rows], start=True, stop=True,
                )

            for _ in range(11):
                warm(wsrc, 256)

            # ---- SIREN layer 1: h1 = sin(2pi*v1 - pi) on all 128 partitions
            h1 = sb.tile([128, 128], F16)
            nc.scalar.activation(
                h1[:], v1, mybir.ActivationFunctionType.Sin, scale=TWO_PI
            )

            # ---- SIREN layer 2: 4 concurrent 32x32 tile_position matmuls
            # (W2 pre-scaled by omega/2pi on host), then one fused
            # (+c2' mod 1) on DVE and the Sin ACT.
            mm2 = ps.tile([128, 128], F32, tag="mm2")
            for jg in range(NJB):
                s = slice(32 * jg, 32 * jg + 32)
                nc.tensor.matmul(
                    mm2[s, :], w2h[s, :], h1[s, :],
                    start=True, stop=True, tile_position=(32 * jg, 32 * jg),
                )
            # gap fillers: anchored by a data dependency on h1/kr so the
            # scheduler cannot float them ahead of mm2; under-sized so they
            # end before layer 3 becomes ready.  They keep the PE's DVFS
            # ramp alive through the DVE/ACT chain.
            warm(h1, 64)
            warm(h1, 64)
            warm(h1, 64)
            warm(h1, 64)
            t1 = sb.tile([128, 128], F32)
            nc.vector.tensor_scalar(
                t1[:], mm2[:], c2, MAGIC,
                mybir.AluOpType.add, mybir.AluOpType.add,
            )
            kr = sb.tile([128, 128], F32)
            nc.vector.tensor_scalar(
                kr[:], t1[:], MAGIC, None, mybir.AluOpType.subtract
            )
            warm(kr, 64)
            d2 = sb.tile([128, 128], F32)
            nc.vector.tensor_sub(d2[:], mm2[:], kr[:])
            warm(d2, 48)
            h2 = sb.tile([128, 128], F16)
            nc.scalar.activation(
                h2[:], d2[:], mybir.ActivationFunctionType.Sin,
                bias=c2b, scale=TWO_PI,
            )

            # ---- layer 3: K[jj, m] per lag block b -- 4 concurrent matmuls
            # into one 4-bank PSUM tile (block b at bank b, first 32 cols),
            # then a single strided-AP add of b3 converting to bf16.
            kps = ps.tile([128, NJB * BANK], F32, tag="kps")
            for b in range(NJB):
                s = slice(32 * b, 32 * b + 32)
                nc.tensor.matmul(
                    kps[:, b * BANK : b * BANK + 2 * COUT],
                    h2[s, :], w3h[s, :],
                    start=True, stop=True, tile_position=(32 * b, 0),
                )
            warm(h2, 64)
            warm(h2, 32)
            warm(h2, 128)
            warm(h2, 128)
            # bias add in two halves with SEPARATE destination tiles (tile-
            # granular deps -- slicing one tile races the conv's LDWEIGHTS
            # prefetch on hardware): the conv's first chunks need only
            # blocks 0-1, so they start after the first half.
            b3v2 = (
                pt2[:, P_B3 - P_W2 : P_B3 - P_W2 + 2 * COUT]
                .unsqueeze(1)
                .broadcast_to([128, 2, 2 * COUT])
            )
            kview = kps[:].rearrange("p (b n) -> p b n", b=NJB)
            ksbA = sb.tile([128, 2 * 2 * COUT], BF16)
            nc.vector.tensor_add(
                ksbA[:].rearrange("p (b n) -> p b n", b=2),
                kview[:, 0:2, 0 : 2 * COUT], b3v2,
            )
            ksbB = sb.tile([128, 2 * 2 * COUT], BF16)
            nc.vector.tensor_add(
                ksbB[:].rearrange("p (b n) -> p b n", b=2),
                kview[:, 2:NJB, 0 : 2 * COUT], b3v2,
            )

            # ---- causal conv, split into two accumulation groups by e-half
            # so the first half's PSUM->SBUF copy + DMA overlap the second
            # half's matmuls.  Each group's first chunk covers the whole
            # group range (opens it) and its last chunk does too (closes it).
            # chunk (b, ci) covers e in [128b, 512) (causal trimming); the
            # e-half [e0, e1) slice of it is chunk-cols [e0-128b, e1-128b).
            th = sb.tile([COUT, L], F32)
            halves = [(0, 384, [(0, 0), (0, 1), (0, 2), (1, 2), (1, 1), (1, 0)]),
                      (384, 512, [(0, 0), (0, 1), (0, 2), (0, 3),
                                  (1, 3), (1, 2), (1, 1), (1, 0)])]
            for e0, e1, grp in halves:
                # full-bank tile so the two groups never share a PSUM bank
                # (sharing would serialize group B behind group A's copy)
                vp = ps.tile([COUT, BANK], F32, name=f"vp{e0}", tag=f"vp{e0}")
                for idx, (ci, b) in enumerate(grp):
                    kt = ksbA if b < 2 else ksbB
                    lhs = kt[:, (b % 2) * 2 * COUT + ci * COUT
                             : (b % 2) * 2 * COUT + (ci + 1) * COUT]
                    off = ci * HCOLS_HALF + CH_OFF_A[b] + max(e0 - 128 * b, 0)
                    lo = max(e0, 128 * b)
                    nc.tensor.matmul(
                        vp[0:COUT, lo - e0 : e1 - e0],
                        lhs, ht[:, off : off + (e1 - lo)],
                        start=(idx == 0), stop=(idx == len(grp) - 1),
                    )
                nc.vector.tensor_copy(th[:, e0:e1], vp[0:COUT, 0 : e1 - e0])
                eng = nc.scalar if e0 == 0 else nc.sync
                eng.dma_start(out_d[:, e0:e1], th[:, e0:e1])

    nc.compile()
    return nc


def _host_prep(inputs):
    """Fold params and build per-core in_maps (numpy)."""
    import ml_dtypes

    x = np.asarray(inputs["x"], np.float32)
    t = np.asarray(inputs["t"], np.float32)
    t_eval = np.asarray(inputs["t_eval"], np.float32)
    v1 = np.asarray(inputs["v1"], np.float32)
    g1 = np.asarray(inputs["g1"], np.float32)
    b1 = np.asarray(inputs["b1"], np.float32)
    v2 = np.asarray(inputs["v2"], np.float32)
    g2 = np.asarray(inputs["g2"], np.float32)
    b2 = np.asarray(inputs["b2"], np.float32)
    W3 = np.asarray(inputs["W3"], np.float32)
    b3 = np.asarray(inputs["b3"], np.float32)

    # weight norm (fp32, matching reference)
    W1 = (g1[:, None] * v1 / np.linalg.norm(v1, axis=1, keepdims=True))[:, 0]
    W2 = g2[:, None] * v2 / np.linalg.norm(v2, axis=1, keepdims=True)

    # rel_j = t[0] - t_eval[j]  (== -j/512 exactly on the arange grid)
    rel = (np.float32(t[0]) - t_eval).astype(np.float64)

    s = np.float64(OMEGA) / TWO_PI
    a1 = s * W1.astype(np.float64)
    c1 = s * b1.astype(np.float64)
    # layer-1 argument in cycles, range-reduced on host (pure function of
    # the known time grid + params): sin(2pi*v1) == sin(2pi*u1)
    u1 = a1[:, None] * rel[None, :] + c1[:, None]             # (H, 512)
    v1c = (u1 - np.round(u1)).astype(np.float32)              # (H, 512)

    c2 = (s * b2.astype(np.float64)).astype(np.float32)
    c2b = (np.float64(OMEGA) * b2.astype(np.float64)).astype(np.float32)
    w2s = (s * W2.astype(np.float64)).astype(np.float32)      # (H, H)

    xpad = np.zeros((PAD + L, CIN), np.float32)
    xpad[PAD:] = x

    # shared parts of the packed params (128, PCOLS)
    base = np.zeros((128, PCOLS), np.float32)
    # v-layout: partition p = 32*jg + i covers lags 128jg..128jg+127
    base[:, P_V1 : P_V1 + 128] = (
        v1c.reshape(H, NJB, 128).transpose(1, 0, 2).reshape(128, 128)
    )
    base[:, P_C2] = np.tile(c2, NJB)
    base[:, P_C2B] = np.tile(c2b, NJB)
    base[:, P_W2 : P_W2 + 16] = (
        np.tile(w2s.T, (NJB, 1)).astype(np.float16).view(np.float32)
    )

    in_maps = []
    for m in range(NCORES):
        cols = []
        for ci in range(2):
            c = 2 * m + ci
            cols.extend(g * CIN + c for g in range(COUT))
        params = base.copy()
        params[:, P_W3 : P_W3 + 16] = (
            np.tile(W3[cols, :].T, (NJB, 1)).astype(np.float16).view(np.float32)
        )
        params[:, P_B3 : P_B3 + 2 * COUT] = np.broadcast_to(b3[cols], (128, 2 * COUT))

        hank = np.zeros((128, HCOLS), ml_dtypes.bfloat16)
        for ci in range(2):
            c = 2 * m + ci
            # H[p, e] = x[e - 128*b - p, c] (0 when index < 0)
            w = np.lib.stride_tricks.sliding_window_view(xpad[:, c], L)
            for b in range(NJB):
                rows = PAD - 128 * b - np.arange(128)
                off = ci * HCOLS_HALF + CH_OFF_A[b]
                hank[:, off : off + CH_N[b]] = w[rows][:, 128 * b : L].astype(
                    ml_dtypes.bfloat16
                )
        in_maps.append({"params": params, "hank": hank})
    return in_maps


def kernel(**inputs) -> np.ndarray:
    if "nc" not in _CACHE:
        _CACHE["nc"] = _build_module()
    nc = _CACHE["nc"]
    in_maps = _host_prep(inputs)
    res = run_bass_kernel_spmd(nc, in_maps, list(range(NCORES)))
    partial = np.zeros((COUT, L), np.float64)
    for r in res.results:
        partial += r["out"].astype(np.float64)
    return partial.T.astype(np.float32)
